# revision 18
# baseline (speedup 1.0000x reference)
"""Trainium2 Bass kernel for nn_BiologicalNormalization.

Math: three chained per-sample LayerNorms (affine params gathered per-sample
by id on the host). The trailing gated blend ``x*sigmoid(xW+b) +
x*(1-sigmoid(xW+b))`` is mathematically the identity, so the kernel returns
the triple-LayerNorm result directly.

Distribution: pure data parallelism - batch 2048 is split into 8 shards of
256 samples, one per NeuronCore. Per-id affine tables are gathered to
per-sample rows on the host (tiny), so each core only sees dense tensors.

Per-core schedule (partition dim = 128 samples, free dim = D=512, sequence
positions in chunks of K=8). The previous revision funneled ~61us/chunk of
work through the Vector engine alone; this version splits each LayerNorm's
work across four engines so the per-chunk critical path is ~20us:

  - Vector (DVE): x cast+row-sums (tensor_scalar w/ accum, 2x mode),
    gamma multiplies (K-fused tensor_tensor, bf16 2x), beta adds fused with
    row-sums (scalar_tensor_tensor w/ accum), LN3 centering (tensor_scalar
    two-op w/ per-partition scalar APs), stats finalization smalls.
  - Scalar (ACT): sum-of-squares via Square activations with accum_out, and
    the Sqrt in rstd finalization (both functions live in the same
    activation table set, so no table reloads).
  - GPSIMD: LN1/LN2 centering via tensor_scalar two-op (1-input streaming).
  - DMA: f32 input, bf16 output (output upcast to f32 on host).

Stats use raw sums (V = D*Sum(y^2) - Sum(y)^2 = D^2*var) so the per-(row,
position) rstd/mean finalize in a handful of small [128,8] ops. A 7-stage
software pipeline across 32 chunks keeps all engines busy. Intermediates are
bf16 (rel-err budget 2e-2; measured ~7e-3); statistics accumulate in f32.
"""

import contextlib

import ml_dtypes
import numpy as np

import concourse.bass as bass
import concourse.bacc as bacc
import concourse.mybir as mybir
from concourse.tile import TileContext

NCORES = 8
B, S, D = 2048, 128, 512
BS = B // NCORES  # samples per core
P = 128  # SBUF partitions (samples per group)
NGRP = BS // P
K = 8  # sequence positions per chunk
EPS = 1e-5
FP = mybir.dt.float32
BF = mybir.dt.bfloat16
PARAM_NAMES = ("g1", "b1", "g2", "b2", "g3", "b3")

SUB = mybir.AluOpType.subtract
MUL = mybir.AluOpType.mult
ADD = mybir.AluOpType.add
SQUARE = mybir.ActivationFunctionType.Square
SQRT = mybir.ActivationFunctionType.Sqrt


def _bcast_mid(t, k):
    """[P, D] param tile -> [P, k, D] AP, 0-stride on the middle dim."""
    return bass.AP(tensor=t.tensor, offset=t.offset, ap=[t.ap[0], [0, k], t.ap[1]])


def _build(repeat=1):
    nc = bacc.Bacc("TRN2", target_bir_lowering=False, debug=False, num_devices=NCORES)
    x = nc.declare_dram_parameter("x", [BS, S, D], BF, isOutput=False).ap()
    prm = {
        k: nc.declare_dram_parameter(k, [BS, D], BF, isOutput=False).ap()
        for k in PARAM_NAMES
    }
    out = nc.declare_dram_parameter("out", [BS, S, D], BF, isOutput=True).ap()

    with TileContext(nc) as tc:
        with contextlib.ExitStack() as stack:
            pp = stack.enter_context(tc.tile_pool(name="params", bufs=2))
            px = stack.enter_context(tc.tile_pool(name="xin", bufs=4))
            pz = stack.enter_context(tc.tile_pool(name="zpool", bufs=5))
            pu = stack.enter_context(tc.tile_pool(name="upool", bufs=4))
            py = stack.enter_context(tc.tile_pool(name="ypool", bufs=5))
            po = stack.enter_context(tc.tile_pool(name="yout", bufs=2))
            pdmp = stack.enter_context(tc.tile_pool(name="dumps", bufs=2))
            ps = stack.enter_context(tc.tile_pool(name="small", bufs=4))
            pc = stack.enter_context(tc.tile_pool(name="singles", bufs=1))
            eps_tile = pc.tile([P, 1], FP)
            nc.vector.memset(eps_tile, EPS * D * D)

            def finish(s, q, tag):
                """[P, K] raw sums -> (r, mrneg) for z = y*r + mrneg.
                V = D*q - s^2 = D^2*var, rp = 1/sqrt(V + eps*D^2) = r/D,
                mrneg = -s*rp = -mean*r, r = D*rp."""
                a = ps.tile([P, K], FP, tag=f"a{tag}")
                nc.vector.tensor_tensor(out=a, in0=s, in1=s, op=MUL)
                V = ps.tile([P, K], FP, tag=f"V{tag}")
                nc.vector.scalar_tensor_tensor(
                    out=V, in0=q, scalar=float(D), in1=a, op0=MUL, op1=SUB
                )
                std = ps.tile([P, K], FP, tag=f"std{tag}")
                nc.scalar.activation(out=std, in_=V, func=SQRT, bias=eps_tile)
                rp = ps.tile([P, K], FP, tag=f"rp{tag}")
                nc.vector.reciprocal(out=rp, in_=std)
                mrneg = ps.tile([P, K], FP, tag=f"mr{tag}")
                nc.vector.scalar_tensor_tensor(
                    out=mrneg, in0=s, scalar=-1.0, in1=rp, op0=MUL, op1=MUL
                )
                r = ps.tile([P, K], FP, tag=f"r{tag}")
                nc.vector.tensor_scalar_mul(out=r, in0=rp, scalar1=float(D))
                return r, mrneg

            def s0_load(st):
                b0, s0 = st["b0"], st["s0"]
                xt = px.tile([P, K, D], BF)
                nc.sync.dma_start(out=xt, in_=x[b0 : b0 + P, s0 : s0 + K, :])
                st["xt"] = xt

            def s1_statsx(st):
                xt = st["xt"]
                s1t = ps.tile([P, K], FP, tag="s1")
                q1t = ps.tile([P, K], FP, tag="q1")
                dumpA = pdmp.tile([P, D], BF, tag="sqA")
                for k in range(K):
                    # row sums on DVE (out is a throwaway copy); the Pool
                    # engine cannot host the accum_out variant (ISA check)
                    nc.vector.tensor_scalar(
                        out=dumpA, in0=xt[:, k, :],
                        scalar1=1.0, scalar2=None, op0=MUL, op1=ADD,
                        accum_out=s1t[:, k : k + 1],
                    )
                dumpB = pdmp.tile([P, D], BF, tag="sqB")
                for k in range(K):
                    # sum of squares on ScalarE
                    nc.scalar.activation(
                        out=dumpB, in_=xt[:, k, :], func=SQUARE,
                        accum_out=q1t[:, k : k + 1],
                    )
                st["s1"], st["q1"] = s1t, q1t

            def s2_f1(st):
                st["rm1"] = finish(st["s1"], st["q1"], "1")

            def s2b_c1(st):
                r1, mr1 = st["rm1"]
                xt = st["xt"]
                z1 = pz.tile([P, K, D], BF, tag="z")
                for k in range(K):
                    nc.gpsimd.tensor_scalar(
                        out=z1[:, k, :], in0=xt[:, k, :],
                        scalar1=r1[:, k : k + 1], scalar2=mr1[:, k : k + 1],
                        op0=MUL, op1=ADD,
                    )
                st["z1"] = z1

            def s3_ln1(st):
                z1, pt = st["z1"], st["pt"]
                u1 = pu.tile([P, K, D], BF, tag="u")
                nc.vector.tensor_tensor(
                    out=u1, in0=z1, in1=_bcast_mid(pt["g1"], K), op=MUL
                )
                y1 = py.tile([P, K, D], BF, tag="y")
                s2t = ps.tile([P, K], FP, tag="s2")
                q2t = ps.tile([P, K], FP, tag="q2")
                for k in range(K):
                    nc.vector.scalar_tensor_tensor(
                        out=y1[:, k, :], in0=u1[:, k, :], scalar=1.0,
                        in1=pt["b1"], op0=MUL, op1=ADD,
                        accum_out=s2t[:, k : k + 1],
                    )
                dump1 = pdmp.tile([P, D], BF, tag="sq1")
                for k in range(K):
                    nc.scalar.activation(
                        out=dump1, in_=y1[:, k, :], func=SQUARE,
                        accum_out=q2t[:, k : k + 1],
                    )
                st["y1"], st["s2"], st["q2"] = y1, s2t, q2t

            def s4_f2(st):
                st["rm2"] = finish(st["s2"], st["q2"], "2")

            def s4b_c2(st):
                r2, mr2 = st["rm2"]
                y1 = st["y1"]
                z2 = pz.tile([P, K, D], BF, tag="z")
                for k in range(K):
                    nc.gpsimd.tensor_scalar(
                        out=z2[:, k, :], in0=y1[:, k, :],
                        scalar1=r2[:, k : k + 1], scalar2=mr2[:, k : k + 1],
                        op0=MUL, op1=ADD,
                    )
                st["z2"] = z2

            def s5_ln2(st):
                z2, pt = st["z2"], st["pt"]
                u2 = pu.tile([P, K, D], BF, tag="u")
                nc.vector.tensor_tensor(
                    out=u2, in0=z2, in1=_bcast_mid(pt["g2"], K), op=MUL
                )
                y2 = py.tile([P, K, D], BF, tag="y")
                s3t = ps.tile([P, K], FP, tag="s3")
                q3t = ps.tile([P, K], FP, tag="q3")
                for k in range(K):
                    nc.vector.scalar_tensor_tensor(
                        out=y2[:, k, :], in0=u2[:, k, :], scalar=1.0,
                        in1=pt["b2"], op0=MUL, op1=ADD,
                        accum_out=s3t[:, k : k + 1],
                    )
                dump2 = pdmp.tile([P, D], BF, tag="sq2")
                for k in range(K):
                    nc.scalar.activation(
                        out=dump2, in_=y2[:, k, :], func=SQUARE,
                        accum_out=q3t[:, k : k + 1],
                    )
                st["y2"], st["s3"], st["q3"] = y2, s3t, q3t

            def s6_f3(st):
                st["rm3"] = finish(st["s3"], st["q3"], "3")

            def s6b_ln3(st):
                b0, s0 = st["b0"], st["s0"]
                pt = st["pt"]
                r3, mr3 = st["rm3"]
                y2 = st["y2"]
                z3 = pz.tile([P, K, D], BF, tag="z")
                for k in range(K):
                    nc.gpsimd.tensor_scalar(
                        out=z3[:, k, :], in0=y2[:, k, :],
                        scalar1=r3[:, k : k + 1], scalar2=mr3[:, k : k + 1],
                        op0=MUL, op1=ADD,
                    )
                u3 = pu.tile([P, K, D], BF, tag="u")
                nc.vector.tensor_tensor(
                    out=u3, in0=z3, in1=_bcast_mid(pt["g3"], K), op=MUL
                )
                ot = po.tile([P, K, D], BF)
                nc.gpsimd.tensor_tensor(
                    out=ot, in0=u3, in1=_bcast_mid(pt["b3"], K), op=ADD
                )
                nc.sync.dma_start(out=out[b0 : b0 + P, s0 : s0 + K, :], in_=ot)

            STAGES = [
                s0_load, s1_statsx, s2_f1, s2b_c1, s3_ln1,
                s4_f2, s4b_c2, s5_ln2, s6_f3, s6b_ln3,
            ]

            def body():
                pts = []
                for grp in range(NGRP):
                    b0 = grp * P
                    pt = {}
                    for kname in PARAM_NAMES:
                        t = pp.tile([P, D], BF, tag=kname)
                        nc.sync.dma_start(out=t, in_=prm[kname][b0 : b0 + P, :])
                        pt[kname] = t
                    pts.append(pt)
                chunks = [
                    {"pt": pts[grp], "b0": grp * P, "s0": c * K}
                    for c in range(S // K)
                    for grp in range(NGRP)
                ]
                n = len(chunks)
                depth = len(STAGES)
                for i in range(n + depth - 1):
                    for d in reversed(range(depth)):
                        ci = i - d
                        if 0 <= ci < n:
                            STAGES[d](chunks[ci])
                for st in chunks:
                    st.clear()

            if repeat == 1:
                body()
            else:
                with tc.For_i(0, repeat, 1):
                    body()
    nc.compile()
    return nc



class _Runner:
    """Persistent compiled SPMD executor for the Bass graph.

    Mirrors bass2jax.run_bass_via_pjrt but keeps the jitted callable and the
    device mesh alive so repeated calls don't retrace/recompile.
    """

    def __init__(self, nc):
        import jax
        import concourse.bass2jax as bass2jax
        from jax.experimental.shard_map import shard_map
        from jax.sharding import Mesh, NamedSharding, PartitionSpec

        bass2jax.install_neuronx_cc_hook()
        self._jax = jax
        self._nc = nc

        partition_name = (
            nc.partition_id_tensor.name if nc.partition_id_tensor else None
        )
        in_names = []
        out_names = []
        out_avals = []
        for alloc in nc.m.functions[0].allocations:
            if not isinstance(alloc, mybir.MemoryLocationSet):
                continue
            name = alloc.memorylocations[0].name
            if alloc.kind == "ExternalInput":
                if name != partition_name:
                    in_names.append(name)
            elif alloc.kind == "ExternalOutput":
                out_names.append(name)
                out_avals.append(
                    jax.core.ShapedArray(
                        tuple(alloc.tensor_shape), mybir.dt.np(alloc.dtype)
                    )
                )
        self.in_names = list(in_names)
        self.out_names = out_names
        self.out_avals = out_avals
        n_params = len(in_names)
        all_in_names = in_names + out_names
        if partition_name is not None:
            all_in_names = all_in_names + [partition_name]

        def _body(*args):
            operands = list(args)
            if partition_name is not None:
                operands.append(bass2jax.partition_id_tensor())
            outs = bass2jax._bass_exec_p.bind(
                *operands,
                out_avals=tuple(out_avals),
                in_names=tuple(all_in_names),
                out_names=tuple(out_names),
                lowering_input_output_aliases=(),
                sim_require_finite=True,
                sim_require_nnan=True,
                nc=nc,
            )
            return tuple(outs)

        devices = jax.devices()[:NCORES]
        self.mesh = Mesh(np.asarray(devices), ("core",))
        self.sharding = NamedSharding(self.mesh, PartitionSpec("core"))
        n_outs = len(out_names)
        donate = tuple(range(n_params, n_params + n_outs))
        self._exec = jax.jit(
            shard_map(
                _body,
                mesh=self.mesh,
                in_specs=(PartitionSpec("core"),) * (n_params + n_outs),
                out_specs=(PartitionSpec("core"),) * n_outs,
                check_rep=False,
            ),
            donate_argnums=donate,
            keep_unused=True,
        )

        def _mk_zeros():
            import jax.numpy as jnp

            return tuple(
                jnp.zeros((NCORES * a.shape[0], *a.shape[1:]), a.dtype)
                for a in out_avals
            )

        self._zeros = jax.jit(
            _mk_zeros, out_shardings=(self.sharding,) * n_outs
        )

    def put_inputs(self, concat_ins):
        """Transfer concatenated (axis0 = NCORES*shard) inputs to devices."""
        return [
            self._jax.device_put(v, self.sharding) for v in concat_ins
        ]

    def run(self, dev_ins):
        """One execution; returns tuple of global output arrays (device)."""
        zeros = self._zeros()
        return self._exec(*dev_ins, *zeros)


_RUNNERS = {}


def get_runner(repeat=1):
    if repeat not in _RUNNERS:
        _RUNNERS[repeat] = _Runner(_build(repeat=repeat))
    return _RUNNERS[repeat]


def host_inputs(
    x,
    pathway_ids,
    compartment_ids,
    cell_type_ids,
    pathway_gamma,
    pathway_beta,
    compartment_gamma,
    compartment_beta,
    cell_type_gamma,
    cell_type_beta,
):
    """Gather per-sample affine rows and cast to the device dtypes."""
    pid = np.asarray(pathway_ids).astype(np.int64)
    cid = np.asarray(compartment_ids).astype(np.int64)
    tid = np.asarray(cell_type_ids).astype(np.int64)
    full = {
        "x": np.ascontiguousarray(
            np.asarray(x, dtype=np.float32).astype(ml_dtypes.bfloat16)
        ),
        "g1": np.asarray(pathway_gamma, np.float32)[pid],
        "b1": np.asarray(pathway_beta, np.float32)[pid],
        "g2": np.asarray(compartment_gamma, np.float32)[cid],
        "b2": np.asarray(compartment_beta, np.float32)[cid],
        "g3": np.asarray(cell_type_gamma, np.float32)[tid],
        "b3": np.asarray(cell_type_beta, np.float32)[tid],
    }
    for k in PARAM_NAMES:
        full[k] = np.ascontiguousarray(full[k].astype(ml_dtypes.bfloat16))
    return full


def kernel(
    x,
    pathway_ids,
    compartment_ids,
    cell_type_ids,
    pathway_gamma,
    pathway_beta,
    compartment_gamma,
    compartment_beta,
    cell_type_gamma,
    cell_type_beta,
    W=None,
    b=None,
    **_unused,
):
    full = host_inputs(
        x,
        pathway_ids,
        compartment_ids,
        cell_type_ids,
        pathway_gamma,
        pathway_beta,
        compartment_gamma,
        compartment_beta,
        cell_type_gamma,
        cell_type_beta,
    )
    runner = get_runner()
    concat_ins = [full[name] for name in runner.in_names]
    dev_ins = runner.put_inputs(concat_ins)
    outs = runner.run(dev_ins)
    return np.asarray(outs[0]).astype(np.float32)


# revision 19
# speedup vs baseline: 1.3196x; 1.3196x over previous
"""Trainium2 Bass kernel for nn_BiologicalNormalization.

Math: three chained per-sample LayerNorms (affine params gathered per-sample
by id on the host). The trailing gated blend ``x*sigmoid(xW+b) +
x*(1-sigmoid(xW+b))`` is mathematically the identity, so the kernel returns
the triple-LayerNorm result directly.

Distribution: pure data parallelism - batch 2048 is split into 8 shards of
256 samples, one per NeuronCore. Per-id affine tables are gathered to
per-sample rows on the host (tiny), so each core only sees dense tensors.

Per-core schedule (partition dim = 128 samples, free dim = D=512, sequence
positions in chunks of K=8). The previous revision funneled ~61us/chunk of
work through the Vector engine alone; this version splits each LayerNorm's
work across four engines so the per-chunk critical path is ~20us:

  - Vector (DVE): x cast+row-sums (tensor_scalar w/ accum, 2x mode),
    gamma multiplies (K-fused tensor_tensor, bf16 2x), beta adds fused with
    row-sums (scalar_tensor_tensor w/ accum), LN3 centering (tensor_scalar
    two-op w/ per-partition scalar APs), stats finalization smalls.
  - Scalar (ACT): sum-of-squares via Square activations with accum_out, and
    the Sqrt in rstd finalization (both functions live in the same
    activation table set, so no table reloads).
  - GPSIMD: LN1/LN2 centering via tensor_scalar two-op (1-input streaming).
  - DMA: f32 input, bf16 output (output upcast to f32 on host).

Stats use raw sums (V = D*Sum(y^2) - Sum(y)^2 = D^2*var) so the per-(row,
position) rstd/mean finalize in a handful of small [128,8] ops. A 7-stage
software pipeline across 32 chunks keeps all engines busy. Intermediates are
bf16 (rel-err budget 2e-2; measured ~7e-3); statistics accumulate in f32.
"""

import contextlib

import ml_dtypes
import numpy as np

import concourse.bass as bass
import concourse.bacc as bacc
import concourse.mybir as mybir
from concourse.tile import TileContext

NCORES = 8
B, S, D = 2048, 128, 512
BS = B // NCORES  # samples per core
P = 128  # SBUF partitions (samples per group)
NGRP = BS // P
K = 8  # sequence positions per chunk
EPS = 1e-5
FP = mybir.dt.float32
BF = mybir.dt.bfloat16
PARAM_NAMES = ("g1", "b1", "g2", "b2", "g3", "b3")

SUB = mybir.AluOpType.subtract
MUL = mybir.AluOpType.mult
ADD = mybir.AluOpType.add
SQUARE = mybir.ActivationFunctionType.Square
SQRT = mybir.ActivationFunctionType.Sqrt


def _bcast_mid(t, k):
    """[P, D] param tile -> [P, k, D] AP, 0-stride on the middle dim."""
    return bass.AP(tensor=t.tensor, offset=t.offset, ap=[t.ap[0], [0, k], t.ap[1]])


def _build(repeat=1):
    nc = bacc.Bacc("TRN2", target_bir_lowering=False, debug=False, num_devices=NCORES)
    x = nc.declare_dram_parameter("x", [BS, S, D], BF, isOutput=False).ap()
    prm = {
        k: nc.declare_dram_parameter(k, [BS, D], BF, isOutput=False).ap()
        for k in PARAM_NAMES
    }
    out = nc.declare_dram_parameter("out", [BS, S, D], BF, isOutput=True).ap()

    with TileContext(nc) as tc:
        with contextlib.ExitStack() as stack:
            pp = stack.enter_context(tc.tile_pool(name="params", bufs=2))
            px = stack.enter_context(tc.tile_pool(name="xin", bufs=4))
            pz = stack.enter_context(tc.tile_pool(name="zpool", bufs=5))
            pu = stack.enter_context(tc.tile_pool(name="upool", bufs=4))
            py = stack.enter_context(tc.tile_pool(name="ypool", bufs=5))
            po = stack.enter_context(tc.tile_pool(name="yout", bufs=2))
            pdmp = stack.enter_context(tc.tile_pool(name="dumps", bufs=2))
            ps = stack.enter_context(tc.tile_pool(name="small", bufs=4))
            pc = stack.enter_context(tc.tile_pool(name="singles", bufs=1))
            eps_tile = pc.tile([P, 1], FP)
            nc.vector.memset(eps_tile, EPS * D * D)

            def finish(s, q, tag):
                """[P, K] raw sums -> (r, mrneg) for z = y*r + mrneg.
                V = D*q - s^2 = D^2*var, rp = 1/sqrt(V + eps*D^2) = r/D,
                mrneg = -s*rp = -mean*r, r = D*rp."""
                a = ps.tile([P, K], FP, tag=f"a{tag}")
                nc.vector.tensor_tensor(out=a, in0=s, in1=s, op=MUL)
                V = ps.tile([P, K], FP, tag=f"V{tag}")
                nc.vector.scalar_tensor_tensor(
                    out=V, in0=q, scalar=float(D), in1=a, op0=MUL, op1=SUB
                )
                std = ps.tile([P, K], FP, tag=f"std{tag}")
                nc.scalar.activation(out=std, in_=V, func=SQRT, bias=eps_tile)
                rp = ps.tile([P, K], FP, tag=f"rp{tag}")
                nc.vector.reciprocal(out=rp, in_=std)
                mrneg = ps.tile([P, K], FP, tag=f"mr{tag}")
                nc.vector.scalar_tensor_tensor(
                    out=mrneg, in0=s, scalar=-1.0, in1=rp, op0=MUL, op1=MUL
                )
                r = ps.tile([P, K], FP, tag=f"r{tag}")
                nc.vector.tensor_scalar_mul(out=r, in0=rp, scalar1=float(D))
                return r, mrneg

            def s0_load(st):
                b0, s0 = st["b0"], st["s0"]
                xt = px.tile([P, K, D], BF)
                nc.sync.dma_start(out=xt, in_=x[b0 : b0 + P, s0 : s0 + K, :])
                st["xt"] = xt

            def s1_statsx(st):
                xt = st["xt"]
                s1t = ps.tile([P, K], FP, tag="s1")
                q1t = ps.tile([P, K], FP, tag="q1")
                dumpA = pdmp.tile([P, D], BF, tag="sqA")
                for k in range(K):
                    # row sums on DVE (out is a throwaway copy); the Pool
                    # engine cannot host the accum_out variant (ISA check)
                    nc.vector.tensor_scalar(
                        out=dumpA, in0=xt[:, k, :],
                        scalar1=1.0, scalar2=None, op0=MUL, op1=ADD,
                        accum_out=s1t[:, k : k + 1],
                    )
                dumpB = pdmp.tile([P, D], BF, tag="sqB")
                for k in range(K):
                    # sum of squares on ScalarE
                    nc.scalar.activation(
                        out=dumpB, in_=xt[:, k, :], func=SQUARE,
                        accum_out=q1t[:, k : k + 1],
                    )
                st["s1"], st["q1"] = s1t, q1t

            def s2_f1(st):
                st["rm1"] = finish(st["s1"], st["q1"], "1")

            def s2b_c1(st):
                r1, mr1 = st["rm1"]
                xt = st["xt"]
                z1 = pz.tile([P, K, D], BF, tag="z")
                for k in range(K):
                    nc.gpsimd.tensor_scalar(
                        out=z1[:, k, :], in0=xt[:, k, :],
                        scalar1=r1[:, k : k + 1], scalar2=mr1[:, k : k + 1],
                        op0=MUL, op1=ADD,
                    )
                st["z1"] = z1

            def s3_ln1(st):
                z1, pt = st["z1"], st["pt"]
                u1 = pu.tile([P, K, D], BF, tag="u")
                nc.vector.tensor_tensor(
                    out=u1, in0=z1, in1=_bcast_mid(pt["g1"], K), op=MUL
                )
                y1 = py.tile([P, K, D], BF, tag="y")
                s2t = ps.tile([P, K], FP, tag="s2")
                q2t = ps.tile([P, K], FP, tag="q2")
                for k in range(K):
                    nc.vector.scalar_tensor_tensor(
                        out=y1[:, k, :], in0=u1[:, k, :], scalar=1.0,
                        in1=pt["b1"], op0=MUL, op1=ADD,
                        accum_out=s2t[:, k : k + 1],
                    )
                dump1 = pdmp.tile([P, D], BF, tag="sq1")
                for k in range(K):
                    nc.scalar.activation(
                        out=dump1, in_=y1[:, k, :], func=SQUARE,
                        accum_out=q2t[:, k : k + 1],
                    )
                st["y1"], st["s2"], st["q2"] = y1, s2t, q2t

            def s4_f2(st):
                st["rm2"] = finish(st["s2"], st["q2"], "2")

            def s4b_c2(st):
                r2, mr2 = st["rm2"]
                y1 = st["y1"]
                z2 = pz.tile([P, K, D], BF, tag="z")
                for k in range(K):
                    nc.gpsimd.tensor_scalar(
                        out=z2[:, k, :], in0=y1[:, k, :],
                        scalar1=r2[:, k : k + 1], scalar2=mr2[:, k : k + 1],
                        op0=MUL, op1=ADD,
                    )
                st["z2"] = z2

            def s5_ln2(st):
                z2, pt = st["z2"], st["pt"]
                u2 = pu.tile([P, K, D], BF, tag="u")
                nc.vector.tensor_tensor(
                    out=u2, in0=z2, in1=_bcast_mid(pt["g2"], K), op=MUL
                )
                y2 = py.tile([P, K, D], BF, tag="y")
                s3t = ps.tile([P, K], FP, tag="s3")
                q3t = ps.tile([P, K], FP, tag="q3")
                for k in range(K):
                    nc.vector.scalar_tensor_tensor(
                        out=y2[:, k, :], in0=u2[:, k, :], scalar=1.0,
                        in1=pt["b2"], op0=MUL, op1=ADD,
                        accum_out=s3t[:, k : k + 1],
                    )
                dump2 = pdmp.tile([P, D], BF, tag="sq2")
                for k in range(K):
                    nc.scalar.activation(
                        out=dump2, in_=y2[:, k, :], func=SQUARE,
                        accum_out=q3t[:, k : k + 1],
                    )
                st["y2"], st["s3"], st["q3"] = y2, s3t, q3t

            def s6_f3(st):
                st["rm3"] = finish(st["s3"], st["q3"], "3")

            def s6b_ln3(st):
                b0, s0 = st["b0"], st["s0"]
                pt = st["pt"]
                r3, mr3 = st["rm3"]
                y2 = st["y2"]
                z3 = pz.tile([P, K, D], BF, tag="z")
                for k in range(K):
                    nc.gpsimd.tensor_scalar(
                        out=z3[:, k, :], in0=y2[:, k, :],
                        scalar1=r3[:, k : k + 1], scalar2=mr3[:, k : k + 1],
                        op0=MUL, op1=ADD,
                    )
                u3 = pu.tile([P, K, D], BF, tag="u")
                nc.vector.tensor_tensor(
                    out=u3, in0=z3, in1=_bcast_mid(pt["g3"], K), op=MUL
                )
                ot = po.tile([P, K, D], BF)
                nc.vector.tensor_tensor(
                    out=ot, in0=u3, in1=_bcast_mid(pt["b3"], K), op=ADD
                )
                nc.sync.dma_start(out=out[b0 : b0 + P, s0 : s0 + K, :], in_=ot)

            STAGES = [
                s0_load, s1_statsx, s2_f1, s2b_c1, s3_ln1,
                s4_f2, s4b_c2, s5_ln2, s6_f3, s6b_ln3,
            ]

            def body():
                pts = []
                for grp in range(NGRP):
                    b0 = grp * P
                    pt = {}
                    for kname in PARAM_NAMES:
                        t = pp.tile([P, D], BF, tag=kname)
                        nc.sync.dma_start(out=t, in_=prm[kname][b0 : b0 + P, :])
                        pt[kname] = t
                    pts.append(pt)
                chunks = [
                    {"pt": pts[grp], "b0": grp * P, "s0": c * K}
                    for c in range(S // K)
                    for grp in range(NGRP)
                ]
                n = len(chunks)
                depth = len(STAGES)
                for i in range(n + depth - 1):
                    for d in reversed(range(depth)):
                        ci = i - d
                        if 0 <= ci < n:
                            STAGES[d](chunks[ci])
                for st in chunks:
                    st.clear()

            if repeat == 1:
                body()
            else:
                with tc.For_i(0, repeat, 1):
                    body()
    nc.compile()
    return nc



class _Runner:
    """Persistent compiled SPMD executor for the Bass graph.

    Mirrors bass2jax.run_bass_via_pjrt but keeps the jitted callable and the
    device mesh alive so repeated calls don't retrace/recompile.
    """

    def __init__(self, nc):
        import jax
        import concourse.bass2jax as bass2jax
        from jax.experimental.shard_map import shard_map
        from jax.sharding import Mesh, NamedSharding, PartitionSpec

        bass2jax.install_neuronx_cc_hook()
        self._jax = jax
        self._nc = nc

        partition_name = (
            nc.partition_id_tensor.name if nc.partition_id_tensor else None
        )
        in_names = []
        out_names = []
        out_avals = []
        for alloc in nc.m.functions[0].allocations:
            if not isinstance(alloc, mybir.MemoryLocationSet):
                continue
            name = alloc.memorylocations[0].name
            if alloc.kind == "ExternalInput":
                if name != partition_name:
                    in_names.append(name)
            elif alloc.kind == "ExternalOutput":
                out_names.append(name)
                out_avals.append(
                    jax.core.ShapedArray(
                        tuple(alloc.tensor_shape), mybir.dt.np(alloc.dtype)
                    )
                )
        self.in_names = list(in_names)
        self.out_names = out_names
        self.out_avals = out_avals
        n_params = len(in_names)
        all_in_names = in_names + out_names
        if partition_name is not None:
            all_in_names = all_in_names + [partition_name]

        def _body(*args):
            operands = list(args)
            if partition_name is not None:
                operands.append(bass2jax.partition_id_tensor())
            outs = bass2jax._bass_exec_p.bind(
                *operands,
                out_avals=tuple(out_avals),
                in_names=tuple(all_in_names),
                out_names=tuple(out_names),
                lowering_input_output_aliases=(),
                sim_require_finite=True,
                sim_require_nnan=True,
                nc=nc,
            )
            return tuple(outs)

        devices = jax.devices()[:NCORES]
        self.mesh = Mesh(np.asarray(devices), ("core",))
        self.sharding = NamedSharding(self.mesh, PartitionSpec("core"))
        n_outs = len(out_names)
        donate = tuple(range(n_params, n_params + n_outs))
        self._exec = jax.jit(
            shard_map(
                _body,
                mesh=self.mesh,
                in_specs=(PartitionSpec("core"),) * (n_params + n_outs),
                out_specs=(PartitionSpec("core"),) * n_outs,
                check_rep=False,
            ),
            donate_argnums=donate,
            keep_unused=True,
        )

        def _mk_zeros():
            import jax.numpy as jnp

            return tuple(
                jnp.zeros((NCORES * a.shape[0], *a.shape[1:]), a.dtype)
                for a in out_avals
            )

        self._zeros = jax.jit(
            _mk_zeros, out_shardings=(self.sharding,) * n_outs
        )

    def put_inputs(self, concat_ins):
        """Transfer concatenated (axis0 = NCORES*shard) inputs to devices."""
        return [
            self._jax.device_put(v, self.sharding) for v in concat_ins
        ]

    def run(self, dev_ins):
        """One execution; returns tuple of global output arrays (device)."""
        zeros = self._zeros()
        return self._exec(*dev_ins, *zeros)


_RUNNERS = {}


def get_runner(repeat=1):
    if repeat not in _RUNNERS:
        _RUNNERS[repeat] = _Runner(_build(repeat=repeat))
    return _RUNNERS[repeat]


def host_inputs(
    x,
    pathway_ids,
    compartment_ids,
    cell_type_ids,
    pathway_gamma,
    pathway_beta,
    compartment_gamma,
    compartment_beta,
    cell_type_gamma,
    cell_type_beta,
):
    """Gather per-sample affine rows and cast to the device dtypes."""
    pid = np.asarray(pathway_ids).astype(np.int64)
    cid = np.asarray(compartment_ids).astype(np.int64)
    tid = np.asarray(cell_type_ids).astype(np.int64)
    full = {
        "x": np.ascontiguousarray(
            np.asarray(x, dtype=np.float32).astype(ml_dtypes.bfloat16)
        ),
        "g1": np.asarray(pathway_gamma, np.float32)[pid],
        "b1": np.asarray(pathway_beta, np.float32)[pid],
        "g2": np.asarray(compartment_gamma, np.float32)[cid],
        "b2": np.asarray(compartment_beta, np.float32)[cid],
        "g3": np.asarray(cell_type_gamma, np.float32)[tid],
        "b3": np.asarray(cell_type_beta, np.float32)[tid],
    }
    for k in PARAM_NAMES:
        full[k] = np.ascontiguousarray(full[k].astype(ml_dtypes.bfloat16))
    return full


def kernel(
    x,
    pathway_ids,
    compartment_ids,
    cell_type_ids,
    pathway_gamma,
    pathway_beta,
    compartment_gamma,
    compartment_beta,
    cell_type_gamma,
    cell_type_beta,
    W=None,
    b=None,
    **_unused,
):
    full = host_inputs(
        x,
        pathway_ids,
        compartment_ids,
        cell_type_ids,
        pathway_gamma,
        pathway_beta,
        compartment_gamma,
        compartment_beta,
        cell_type_gamma,
        cell_type_beta,
    )
    runner = get_runner()
    concat_ins = [full[name] for name in runner.in_names]
    dev_ins = runner.put_inputs(concat_ins)
    outs = runner.run(dev_ins)
    return np.asarray(outs[0]).astype(np.float32)


# revision 20
# speedup vs baseline: 1.4925x; 1.1310x over previous
"""Trainium2 Bass kernel for nn_BiologicalNormalization.

Math: three chained per-sample LayerNorms (affine params gathered per-sample
by id on the host). The trailing gated blend ``x*sigmoid(xW+b) +
x*(1-sigmoid(xW+b))`` is mathematically the identity, so the kernel returns
the triple-LayerNorm result directly.

Distribution: pure data parallelism - batch 2048 is split into 8 shards of
256 samples, one per NeuronCore. Per-id affine tables are gathered to
per-sample rows on the host (tiny), so each core only sees dense tensors.

Per-core schedule (partition dim = 128 samples, free dim = D=512, sequence
positions in chunks of K=8). The previous revision funneled ~61us/chunk of
work through the Vector engine alone; this version splits each LayerNorm's
work across four engines so the per-chunk critical path is ~20us:

  - Vector (DVE): x cast+row-sums (tensor_scalar w/ accum, 2x mode),
    gamma multiplies (K-fused tensor_tensor, bf16 2x), beta adds fused with
    row-sums (scalar_tensor_tensor w/ accum), LN3 centering (tensor_scalar
    two-op w/ per-partition scalar APs), stats finalization smalls.
  - Scalar (ACT): sum-of-squares via Square activations with accum_out, and
    the Sqrt in rstd finalization (both functions live in the same
    activation table set, so no table reloads).
  - GPSIMD: LN1/LN2 centering via tensor_scalar two-op (1-input streaming).
  - DMA: f32 input, bf16 output (output upcast to f32 on host).

Stats use raw sums (V = D*Sum(y^2) - Sum(y)^2 = D^2*var) so the per-(row,
position) rstd/mean finalize in a handful of small [128,8] ops. A 7-stage
software pipeline across 32 chunks keeps all engines busy. Intermediates are
bf16 (rel-err budget 2e-2; measured ~7e-3); statistics accumulate in f32.
"""

import contextlib

import ml_dtypes
import numpy as np

import concourse.bass as bass
import concourse.bacc as bacc
import concourse.mybir as mybir
from concourse.tile import TileContext

NCORES = 8
B, S, D = 2048, 128, 512
BS = B // NCORES  # samples per core
P = 128  # SBUF partitions (samples per group)
NGRP = BS // P
K = 8  # sequence positions per chunk
EPS = 1e-5
FP = mybir.dt.float32
BF = mybir.dt.bfloat16
PARAM_NAMES = ("g1", "b1", "g2", "b2", "g3", "b3")

SUB = mybir.AluOpType.subtract
MUL = mybir.AluOpType.mult
ADD = mybir.AluOpType.add
SQUARE = mybir.ActivationFunctionType.Square
SQRT = mybir.ActivationFunctionType.Sqrt


def _bcast_mid(t, k):
    """[P, D] param tile -> [P, k, D] AP, 0-stride on the middle dim."""
    return bass.AP(tensor=t.tensor, offset=t.offset, ap=[t.ap[0], [0, k], t.ap[1]])


def _build(repeat=1):
    nc = bacc.Bacc("TRN2", target_bir_lowering=False, debug=False, num_devices=NCORES)
    x = nc.declare_dram_parameter("x", [BS, S, D], BF, isOutput=False).ap()
    prm = {
        k: nc.declare_dram_parameter(k, [BS, D], BF, isOutput=False).ap()
        for k in PARAM_NAMES
    }
    out = nc.declare_dram_parameter("out", [BS, S, D], BF, isOutput=True).ap()

    with TileContext(nc) as tc:
        with contextlib.ExitStack() as stack:
            pp = stack.enter_context(tc.tile_pool(name="params", bufs=2))
            px = stack.enter_context(tc.tile_pool(name="xin", bufs=4))
            pz = stack.enter_context(tc.tile_pool(name="zpool", bufs=5))
            pu = stack.enter_context(tc.tile_pool(name="upool", bufs=4))
            py = stack.enter_context(tc.tile_pool(name="ypool", bufs=5))
            po = stack.enter_context(tc.tile_pool(name="yout", bufs=2))
            pdmp = stack.enter_context(tc.tile_pool(name="dumps", bufs=2))
            ps = stack.enter_context(tc.tile_pool(name="small", bufs=4))
            pc = stack.enter_context(tc.tile_pool(name="singles", bufs=1))
            eps_tile = pc.tile([P, 1], FP)
            nc.vector.memset(eps_tile, EPS * D * D)

            def finish(s, q, tag):
                """[P, K] raw sums -> (r, mrneg) for z = y*r + mrneg.
                V = D*q - s^2 = D^2*var, rp = 1/sqrt(V + eps*D^2) = r/D,
                mrneg = -s*rp = -mean*r, r = D*rp."""
                a = ps.tile([P, K], FP, tag=f"a{tag}")
                nc.vector.tensor_tensor(out=a, in0=s, in1=s, op=MUL)
                V = ps.tile([P, K], FP, tag=f"V{tag}")
                nc.vector.scalar_tensor_tensor(
                    out=V, in0=q, scalar=float(D), in1=a, op0=MUL, op1=SUB
                )
                std = ps.tile([P, K], FP, tag=f"std{tag}")
                nc.scalar.activation(out=std, in_=V, func=SQRT, bias=eps_tile)
                rp = ps.tile([P, K], FP, tag=f"rp{tag}")
                nc.vector.reciprocal(out=rp, in_=std)
                mrneg = ps.tile([P, K], FP, tag=f"mr{tag}")
                nc.vector.scalar_tensor_tensor(
                    out=mrneg, in0=s, scalar=-1.0, in1=rp, op0=MUL, op1=MUL
                )
                r = ps.tile([P, K], FP, tag=f"r{tag}")
                nc.vector.tensor_scalar_mul(out=r, in0=rp, scalar1=float(D))
                return r, mrneg

            def s0_load(st):
                b0, s0 = st["b0"], st["s0"]
                xt = px.tile([P, K, D], BF)
                nc.sync.dma_start(out=xt, in_=x[b0 : b0 + P, s0 : s0 + K, :])
                st["xt"] = xt

            def s1_statsx(st):
                xt = st["xt"]
                s1t = ps.tile([P, K], FP, tag="s1")
                q1t = ps.tile([P, K], FP, tag="q1")
                dumpA = pdmp.tile([P, D], BF, tag="sqA")
                for k in range(K):
                    # row sums on DVE (out is a throwaway copy); the Pool
                    # engine cannot host the accum_out variant (ISA check)
                    nc.vector.tensor_scalar(
                        out=dumpA, in0=xt[:, k, :],
                        scalar1=1.0, scalar2=None, op0=MUL, op1=ADD,
                        accum_out=s1t[:, k : k + 1],
                    )
                dumpB = pdmp.tile([P, D], BF, tag="sqB")
                for k in range(K):
                    # sum of squares on ScalarE
                    nc.scalar.activation(
                        out=dumpB, in_=xt[:, k, :], func=SQUARE,
                        accum_out=q1t[:, k : k + 1],
                    )
                st["s1"], st["q1"] = s1t, q1t

            def s2_f1(st):
                st["rm1"] = finish(st["s1"], st["q1"], "1")

            def s2b_c1(st):
                r1, mr1 = st["rm1"]
                xt = st["xt"]
                z1 = pz.tile([P, K, D], BF, tag="z")
                for k in range(K):
                    nc.gpsimd.tensor_scalar(
                        out=z1[:, k, :], in0=xt[:, k, :],
                        scalar1=r1[:, k : k + 1], scalar2=mr1[:, k : k + 1],
                        op0=MUL, op1=ADD,
                    )
                st["z1"] = z1

            def s3_ln1(st):
                z1, pt = st["z1"], st["pt"]
                u1 = pu.tile([P, K, D], BF, tag="u")
                nc.vector.tensor_tensor(
                    out=u1, in0=z1, in1=_bcast_mid(pt["g1"], K), op=MUL
                )
                y1 = py.tile([P, K, D], BF, tag="y")
                s2t = ps.tile([P, K], FP, tag="s2")
                q2t = ps.tile([P, K], FP, tag="q2")
                for k in range(K):
                    nc.vector.scalar_tensor_tensor(
                        out=y1[:, k, :], in0=u1[:, k, :], scalar=1.0,
                        in1=pt["b1"], op0=MUL, op1=ADD,
                        accum_out=s2t[:, k : k + 1],
                    )
                dump1 = pdmp.tile([P, D], BF, tag="sq1")
                for k in range(K):
                    nc.scalar.activation(
                        out=dump1, in_=y1[:, k, :], func=SQUARE,
                        accum_out=q2t[:, k : k + 1],
                    )
                st["y1"], st["s2"], st["q2"] = y1, s2t, q2t

            def s4_f2(st):
                st["rm2"] = finish(st["s2"], st["q2"], "2")

            def s4b_c2(st):
                r2, mr2 = st["rm2"]
                y1 = st["y1"]
                z2 = pz.tile([P, K, D], BF, tag="z")
                for k in range(K):
                    nc.gpsimd.tensor_scalar(
                        out=z2[:, k, :], in0=y1[:, k, :],
                        scalar1=r2[:, k : k + 1], scalar2=mr2[:, k : k + 1],
                        op0=MUL, op1=ADD,
                    )
                st["z2"] = z2

            def s5_ln2(st):
                z2, pt = st["z2"], st["pt"]
                u2 = pu.tile([P, K, D], BF, tag="u")
                nc.vector.tensor_tensor(
                    out=u2, in0=z2, in1=_bcast_mid(pt["g2"], K), op=MUL
                )
                y2 = py.tile([P, K, D], BF, tag="y")
                s3t = ps.tile([P, K], FP, tag="s3")
                q3t = ps.tile([P, K], FP, tag="q3")
                for k in range(K):
                    nc.vector.scalar_tensor_tensor(
                        out=y2[:, k, :], in0=u2[:, k, :], scalar=1.0,
                        in1=pt["b2"], op0=MUL, op1=ADD,
                        accum_out=s3t[:, k : k + 1],
                    )
                dump2 = pdmp.tile([P, D], BF, tag="sq2")
                for k in range(K):
                    nc.scalar.activation(
                        out=dump2, in_=y2[:, k, :], func=SQUARE,
                        accum_out=q3t[:, k : k + 1],
                    )
                st["y2"], st["s3"], st["q3"] = y2, s3t, q3t

            def s6_f3(st):
                st["rm3"] = finish(st["s3"], st["q3"], "3")

            def s6b_ln3(st):
                b0, s0 = st["b0"], st["s0"]
                pt = st["pt"]
                r3, mr3 = st["rm3"]
                y2 = st["y2"]
                z3 = pz.tile([P, K, D], BF, tag="z")
                for k in range(K):
                    nc.vector.tensor_scalar(
                        out=z3[:, k, :], in0=y2[:, k, :],
                        scalar1=r3[:, k : k + 1], scalar2=mr3[:, k : k + 1],
                        op0=MUL, op1=ADD,
                    )
                u3 = pu.tile([P, K, D], BF, tag="u")
                nc.vector.tensor_tensor(
                    out=u3, in0=z3, in1=_bcast_mid(pt["g3"], K), op=MUL
                )
                ot = po.tile([P, K, D], BF)
                nc.vector.tensor_tensor(
                    out=ot, in0=u3, in1=_bcast_mid(pt["b3"], K), op=ADD
                )
                nc.sync.dma_start(out=out[b0 : b0 + P, s0 : s0 + K, :], in_=ot)

            STAGES = [
                s0_load, s1_statsx, s2_f1, s2b_c1, s3_ln1,
                s4_f2, s4b_c2, s5_ln2, s6_f3, s6b_ln3,
            ]

            def body():
                pts = []
                for grp in range(NGRP):
                    b0 = grp * P
                    pt = {}
                    for kname in PARAM_NAMES:
                        t = pp.tile([P, D], BF, tag=kname)
                        nc.sync.dma_start(out=t, in_=prm[kname][b0 : b0 + P, :])
                        pt[kname] = t
                    pts.append(pt)
                chunks = [
                    {"pt": pts[grp], "b0": grp * P, "s0": c * K}
                    for c in range(S // K)
                    for grp in range(NGRP)
                ]
                n = len(chunks)
                depth = len(STAGES)
                for i in range(n + depth - 1):
                    for d in reversed(range(depth)):
                        ci = i - d
                        if 0 <= ci < n:
                            STAGES[d](chunks[ci])
                for st in chunks:
                    st.clear()

            if repeat == 1:
                body()
            else:
                with tc.For_i(0, repeat, 1):
                    body()
    nc.compile()
    return nc



class _Runner:
    """Persistent compiled SPMD executor for the Bass graph.

    Mirrors bass2jax.run_bass_via_pjrt but keeps the jitted callable and the
    device mesh alive so repeated calls don't retrace/recompile.
    """

    def __init__(self, nc):
        import jax
        import concourse.bass2jax as bass2jax
        from jax.experimental.shard_map import shard_map
        from jax.sharding import Mesh, NamedSharding, PartitionSpec

        bass2jax.install_neuronx_cc_hook()
        self._jax = jax
        self._nc = nc

        partition_name = (
            nc.partition_id_tensor.name if nc.partition_id_tensor else None
        )
        in_names = []
        out_names = []
        out_avals = []
        for alloc in nc.m.functions[0].allocations:
            if not isinstance(alloc, mybir.MemoryLocationSet):
                continue
            name = alloc.memorylocations[0].name
            if alloc.kind == "ExternalInput":
                if name != partition_name:
                    in_names.append(name)
            elif alloc.kind == "ExternalOutput":
                out_names.append(name)
                out_avals.append(
                    jax.core.ShapedArray(
                        tuple(alloc.tensor_shape), mybir.dt.np(alloc.dtype)
                    )
                )
        self.in_names = list(in_names)
        self.out_names = out_names
        self.out_avals = out_avals
        n_params = len(in_names)
        all_in_names = in_names + out_names
        if partition_name is not None:
            all_in_names = all_in_names + [partition_name]

        def _body(*args):
            operands = list(args)
            if partition_name is not None:
                operands.append(bass2jax.partition_id_tensor())
            outs = bass2jax._bass_exec_p.bind(
                *operands,
                out_avals=tuple(out_avals),
                in_names=tuple(all_in_names),
                out_names=tuple(out_names),
                lowering_input_output_aliases=(),
                sim_require_finite=True,
                sim_require_nnan=True,
                nc=nc,
            )
            return tuple(outs)

        devices = jax.devices()[:NCORES]
        self.mesh = Mesh(np.asarray(devices), ("core",))
        self.sharding = NamedSharding(self.mesh, PartitionSpec("core"))
        n_outs = len(out_names)
        donate = tuple(range(n_params, n_params + n_outs))
        self._exec = jax.jit(
            shard_map(
                _body,
                mesh=self.mesh,
                in_specs=(PartitionSpec("core"),) * (n_params + n_outs),
                out_specs=(PartitionSpec("core"),) * n_outs,
                check_rep=False,
            ),
            donate_argnums=donate,
            keep_unused=True,
        )

        def _mk_zeros():
            import jax.numpy as jnp

            return tuple(
                jnp.zeros((NCORES * a.shape[0], *a.shape[1:]), a.dtype)
                for a in out_avals
            )

        self._zeros = jax.jit(
            _mk_zeros, out_shardings=(self.sharding,) * n_outs
        )

    def put_inputs(self, concat_ins):
        """Transfer concatenated (axis0 = NCORES*shard) inputs to devices."""
        return [
            self._jax.device_put(v, self.sharding) for v in concat_ins
        ]

    def run(self, dev_ins):
        """One execution; returns tuple of global output arrays (device)."""
        zeros = self._zeros()
        return self._exec(*dev_ins, *zeros)


_RUNNERS = {}


def get_runner(repeat=1):
    if repeat not in _RUNNERS:
        _RUNNERS[repeat] = _Runner(_build(repeat=repeat))
    return _RUNNERS[repeat]


def host_inputs(
    x,
    pathway_ids,
    compartment_ids,
    cell_type_ids,
    pathway_gamma,
    pathway_beta,
    compartment_gamma,
    compartment_beta,
    cell_type_gamma,
    cell_type_beta,
):
    """Gather per-sample affine rows and cast to the device dtypes."""
    pid = np.asarray(pathway_ids).astype(np.int64)
    cid = np.asarray(compartment_ids).astype(np.int64)
    tid = np.asarray(cell_type_ids).astype(np.int64)
    full = {
        "x": np.ascontiguousarray(
            np.asarray(x, dtype=np.float32).astype(ml_dtypes.bfloat16)
        ),
        "g1": np.asarray(pathway_gamma, np.float32)[pid],
        "b1": np.asarray(pathway_beta, np.float32)[pid],
        "g2": np.asarray(compartment_gamma, np.float32)[cid],
        "b2": np.asarray(compartment_beta, np.float32)[cid],
        "g3": np.asarray(cell_type_gamma, np.float32)[tid],
        "b3": np.asarray(cell_type_beta, np.float32)[tid],
    }
    for k in PARAM_NAMES:
        full[k] = np.ascontiguousarray(full[k].astype(ml_dtypes.bfloat16))
    return full


def kernel(
    x,
    pathway_ids,
    compartment_ids,
    cell_type_ids,
    pathway_gamma,
    pathway_beta,
    compartment_gamma,
    compartment_beta,
    cell_type_gamma,
    cell_type_beta,
    W=None,
    b=None,
    **_unused,
):
    full = host_inputs(
        x,
        pathway_ids,
        compartment_ids,
        cell_type_ids,
        pathway_gamma,
        pathway_beta,
        compartment_gamma,
        compartment_beta,
        cell_type_gamma,
        cell_type_beta,
    )
    runner = get_runner()
    concat_ins = [full[name] for name in runner.in_names]
    dev_ins = runner.put_inputs(concat_ins)
    outs = runner.run(dev_ins)
    return np.asarray(outs[0]).astype(np.float32)
